# revision 2
# baseline (speedup 1.0000x reference)
"""Trainium2 Bass kernel for nn_DPS_topk (topk_masking) — v8.

Forward output is exactly `hard`: the one-hot expansion of the top-16
indices of (logits + gn) along D, k-axis ordered by ascending index
(see v1 docstring for the stop_gradient cancellation proof).

One-hotness means 1023/1024 of output bytes are zeros, so HBM write
bandwidth binds.  v7 measured: the two HWDGE queues sustain ~440 GB/s
combined at plateau, but lost ~5us stalled on the tile-1 DVE chain
(DVE order was chain0/dense0/chain1/dense1) and ~1us on a late first
write (1 MiB DVE memset on the critical path).

v8 structure per core (256 rows = 2 tiles of 128, out = [256,16K] f32):

  - DVE: small memset (zeros[128,1024], ~0.9us) -> chain0 -> chain1 ->
    dense planes t0 j6..15 -> t1 j6..15.  Chains run back-to-back so
    every dense plane exists long before a queue reaches it.
  - per-plane zero DMAs (one plane = [128 rows, 1024 cols] = 512 KiB):
    tile0 j0..5 + tile1 j0..5 zero-filled; 5 planes on each HWDGE
    queue (sync/scalar), 2 on the gpsimd software queue.
  - ones via gpsimd indirect scatter (4-float granule, one offset per
    partition, ~1.25us desc-gen per plane-call), 12 calls, each gated
    on its plane's zero receipt + the tile's chain (off_sem).
  - dense planes j6..15 per tile (20 total) on DVE via one is_equal
    per plane, DMA'd from the two HWDGE queues gated at production
    index +1 (posted-write slack).
  - inputs: lg+gt0 halves lead each HWDGE queue (chain0 ready ~10us);
    gt1 loads ride the gpsimd software queue.

Raw Bass (no TileContext): one sync-wait condition per instruction;
explicit vector.drain() between dependent same-engine DVE ops; iota on
gpsimd (pattern steps, channel multiplier, base all HW-verified in v7).
"""

import numpy as np

K = 16
D = 1024
N = 64
BS = 32
NCORES = 8
BS_PER_CORE = BS // NCORES   # 4
ROWS = BS_PER_CORE * N       # 256 rows per core
P = 128                      # SBUF partitions
NTILES = ROWS // P           # 2
NELEM = ROWS * K * D
JD = [6, 6]                  # first dense plane per tile (zeros+scatter cover 0..JD-1)
NDENSE = (K - JD[0]) + (K - JD[1])   # 20 dense planes total

_CACHE = {}

# dense planes in DVE production order: t0 j6..15 (incs 1..10), drain (11),
# t1 j6..15 (incs 12..21), final drain (22).
DENSE_ORDER = [(0, j) for j in range(JD[0], K)] + [(1, j) for j in range(JD[1], K)]
ND0 = K - JD[0]
# DMA for a plane waits for the NEXT increment (one-op posted-write slack);
# the last plane of each tile is gated on that tile's drain.
PL_WAIT = {(0, j): (j - JD[0]) + 2 for j in range(JD[0], K - 1)}
PL_WAIT[(0, K - 1)] = ND0 + 1
PL_WAIT.update({(1, j): ND0 + 1 + (j - JD[1]) + 2 for j in range(JD[1], K - 1)})
PL_WAIT[(1, K - 1)] = ND0 + 2 + (K - JD[1])

# zero-plane queue assignment: 5 on sync (A), 5 on scalar (B), 2 on gpsimd (G)
ZPLANES_A = [(0, 0), (0, 2), (0, 4), (1, 1), (1, 3)]
ZPLANES_B = [(0, 1), (0, 3), (0, 5), (1, 0), (1, 2)]
ZPLANES_G = [(1, 4), (1, 5)]
# dense-plane queue assignment (10 each, ascending pl-gate order)
DPLANES_A = [(0, 6), (0, 8), (0, 10), (0, 12), (0, 14),
             (1, 7), (1, 9), (1, 11), (1, 13), (1, 15)]
DPLANES_B = [(0, 7), (0, 9), (0, 11), (0, 13), (0, 15),
             (1, 6), (1, 8), (1, 10), (1, 12), (1, 14)]

# scatter order on gpsimd: tile0 j0..5 then tile1 j0..5.  Gate for (t,j):
# the receipt count on the zero-queue's sem at the position of that plane.
def _zgate(tile, j):
    for sem_name, lst in (("za", ZPLANES_A), ("zb", ZPLANES_B), ("zg", ZPLANES_G)):
        if (tile, j) in lst:
            return sem_name, 16 * (lst.index((tile, j)) + 1)
    raise KeyError((tile, j))


def _build_nc():
    from contextlib import ExitStack

    import concourse.bass as bass
    from concourse import mybir

    f32 = mybir.dt.float32
    i32 = mybir.dt.int32
    u32 = mybir.dt.uint32
    A = mybir.AluOpType

    nc = bass.Bass()
    lg_d = nc.declare_dram_parameter("logits", [N, D], f32, isOutput=False)
    gn_d = nc.declare_dram_parameter("gn", [ROWS, D], f32, isOutput=False)
    out_d = nc.declare_dram_parameter("out", [ROWS, K * D], f32, isOutput=True)

    es = ExitStack()

    def sb(name, shape, dt):
        return es.enter_context(nc.sbuf_tensor(name, shape, dt))

    def sem(name):
        return es.enter_context(nc.semaphore(name))

    zeros = sb("zeros", [P, D], f32)
    gt0 = sb("gt0", [P, D], f32)
    gt1 = sb("gt1", [P, D], f32)
    lg = sb("lg", [P, D], f32)
    x = sb("x", [P, D], f32)
    x2 = sb("x2", [P, D], f32)
    iotaf = sb("iotaf", [P, D], f32)
    chunk = sb("chunk", [P, NDENSE * D], f32)
    v8 = sb("v8", [P, 8], f32)
    v16 = sb("v16", [P, 8], f32)
    i8 = sb("i8", [P, 8], u32)
    i16 = sb("i16", [P, 8], u32)
    sf = sb("sf", [P, K], f32)
    sfr = sb("sfr", [P, K], f32)
    sf2 = [sb("sf2_0", [P, K], f32), sb("sf2_1", [P, K], f32)]
    idx32 = sb("idx32", [P, K], i32)
    al = sb("al", [P, K], i32)
    md = sb("md", [P, K], i32)
    mdf = sb("mdf", [P, K], f32)
    s4f = sb("s4f", [P, 4], f32)
    cm4 = sb("cm4", [P, 1], i32)
    c3 = sb("c3", [P, 1], i32)
    sb_base = [sb("sbase0", [P, K], i32), sb("sbase1", [P, K], i32)]
    offs = [sb("offs0", [P, K], i32), sb("offs1", [P, K], i32)]
    mini = [sb("mini0", [P, 4 * K], f32), sb("mini1", [P, 4 * K], f32)]

    zs_sem = sem("zs_sem")
    in0_sem = sem("in0_sem")
    in1_sem = sem("in1_sem")
    za_sem = sem("za_sem")
    zb_sem = sem("zb_sem")
    zg_sem = sem("zg_sem")
    pda_sem = sem("pda_sem")
    pdb_sem = sem("pdb_sem")
    gp_sem = sem("gp_sem")
    off_sem = sem("off_sem")
    pl_sem = sem("pl_sem")
    sc_sem = sem("sc_sem")
    SEMS = {"za": za_sem, "zb": zb_sem, "zg": zg_sem}

    with nc.Block(no_gpsimd_drain=True) as block:

        def plane_ap(tile, j):
            return bass.AP(out_d, tile * P * K * D + j * D, [[K * D, P], [1, D]])

        def chunk_col(tile, j):
            i = DENSE_ORDER.index((tile, j))
            return chunk[:, i * D : (i + 1) * D]

        @block.sync
        def _(sync: "bass.BassEngine"):
            sync.dma_start(out=lg[0:N, :], in_=lg_d[:, :]).then_inc(in0_sem, 16)
            sync.dma_start(out=gt0[0:N, :], in_=gn_d[0:N, :]).then_inc(in0_sem, 16)
            sync.wait_ge(zs_sem, 1)
            for tile, j in ZPLANES_A:
                sync.dma_start(out=plane_ap(tile, j), in_=zeros[:, :]).then_inc(
                    za_sem, 16
                )
            for tile, j in DPLANES_A:
                sync.wait_ge(pl_sem, PL_WAIT[(tile, j)])
                sync.dma_start(
                    out=plane_ap(tile, j), in_=chunk_col(tile, j)
                ).then_inc(pda_sem, 16)
            sync.wait_ge(in0_sem, 64)
            sync.wait_ge(in1_sem, 32)
            sync.wait_ge(za_sem, 16 * len(ZPLANES_A))
            sync.wait_ge(zb_sem, 16 * len(ZPLANES_B))
            sync.wait_ge(zg_sem, 16 * len(ZPLANES_G))
            sync.wait_ge(pda_sem, 16 * len(DPLANES_A))
            sync.wait_ge(pdb_sem, 16 * len(DPLANES_B))

        @block.scalar
        def _(scalar: "bass.BassEngine"):
            scalar.dma_start(out=lg[N:P, :], in_=lg_d[:, :]).then_inc(in0_sem, 16)
            scalar.dma_start(out=gt0[N:P, :], in_=gn_d[N:P, :]).then_inc(
                in0_sem, 16
            )
            scalar.wait_ge(zs_sem, 1)
            for tile, j in ZPLANES_B:
                scalar.dma_start(out=plane_ap(tile, j), in_=zeros[:, :]).then_inc(
                    zb_sem, 16
                )
            for tile, j in DPLANES_B:
                scalar.wait_ge(pl_sem, PL_WAIT[(tile, j)])
                scalar.dma_start(
                    out=plane_ap(tile, j), in_=chunk_col(tile, j)
                ).then_inc(pdb_sem, 16)

        @block.gpsimd
        def _(gpsimd: "bass.BassEngine"):
            # scatter element offsets: elem(p, slot) =
            #   (tile*128+p)*16384 + (15-slot)*1024 + idx
            for i in range(NTILES):
                gpsimd.iota(
                    sb_base[i][:, :],
                    pattern=[[-D, K]],
                    base=i * P * K * D + (K - 1) * D,
                    channel_multiplier=K * D,
                )
            gpsimd.iota(s4f[:, :], pattern=[[1, 4]], base=0, channel_multiplier=0,
                        allow_small_or_imprecise_dtypes=True)
            gpsimd.iota(iotaf[:, :], pattern=[[1, D]], base=0, channel_multiplier=0,
                        allow_small_or_imprecise_dtypes=True)
            gpsimd.iota(cm4[:, :], pattern=[[1, 1]], base=-4, channel_multiplier=0)
            gpsimd.iota(c3[:, :], pattern=[[1, 1]], base=3, channel_multiplier=0)
            gpsimd.drain().then_inc(gp_sem, 1)

            # gt1 input loads on the software queue
            gpsimd.dma_start(out=gt1[0:N, :], in_=gn_d[P : P + N, :]).then_inc(
                in1_sem, 16
            )
            gpsimd.dma_start(out=gt1[N:P, :], in_=gn_d[P + N : 2 * P, :]).then_inc(
                in1_sem, 16
            )
            # two zero planes as a third bulk stream
            gpsimd.wait_ge(zs_sem, 1)
            for tile, j in ZPLANES_G:
                gpsimd.dma_start(out=plane_ap(tile, j), in_=zeros[:, :]).then_inc(
                    zg_sem, 16
                )

            def scall(tile, s):
                gpsimd.indirect_dma_start(
                    out=bass.AP(out_d, 0, [[1, NELEM], [1, 1]]),
                    out_offset=bass.IndirectOffsetOnAxis(
                        ap=offs[tile][:, s : s + 1], axis=0
                    ),
                    in_=mini[tile][:, 4 * s : 4 * s + 4],
                    in_offset=None,
                ).then_inc(sc_sem, 16)

            # tile-0 planes 0..5 (slots 15..10), then tile-1 planes 0..5
            gpsimd.wait_ge(off_sem, 1)
            for j in range(0, JD[0]):
                gsem, gval = _zgate(0, j)
                gpsimd.wait_ge(SEMS[gsem], gval)
                scall(0, K - 1 - j)
            gpsimd.wait_ge(off_sem, 2)
            for j in range(0, JD[1]):
                gsem, gval = _zgate(1, j)
                gpsimd.wait_ge(SEMS[gsem], gval)
                scall(1, K - 1 - j)
            gpsimd.wait_ge(sc_sem, 16 * (JD[0] + JD[1]))

        @block.vector
        def _(vector: "bass.BassEngine"):
            def dr():
                vector.drain()

            vector.memset(zeros[:], 0.0)
            vector.drain().then_inc(zs_sem, 1)
            vector.wait_ge(gp_sem, 1)

            def chain(tile, gt, in_sem, in_tgt, nmini):
                vector.wait_ge(in_sem, in_tgt)
                vector.tensor_tensor(x[:], gt[:], lg[:], op=A.add)
                dr()
                vector.max(v8[:], x[:])
                dr()
                vector.max_index(i8[:], v8[:], x[:])
                vector.match_replace(x2[:], v8[:], x[:], -1e30)
                dr()
                vector.tensor_copy(sf[:, 0:8], i8[:])
                vector.max(v16[:], x2[:])
                dr()
                vector.max_index(i16[:], v16[:], x2[:])
                dr()
                vector.tensor_copy(sf[:, 8:16], i16[:])
                dr()
                vector.max(sf2[tile][:, 0:8], sf[:])
                dr()
                vector.match_replace(sfr[:], sf2[tile][:, 0:8], sf[:], -1.0)
                dr()
                vector.max(sf2[tile][:, 8:16], sfr[:])
                dr()
                vector.tensor_copy(idx32[:], sf2[tile][:])
                dr()
                vector.tensor_tensor(
                    al[:], idx32[:], cm4[:].to_broadcast([P, K]), op=A.bitwise_and
                )
                vector.tensor_tensor(
                    md[:], idx32[:], c3[:].to_broadcast([P, K]), op=A.bitwise_and
                )
                dr()
                vector.tensor_tensor(
                    offs[tile][:], al[:], sb_base[tile][:], op=A.add
                )
                vector.tensor_copy(mdf[:], md[:])
                dr()
                for s in range(K - nmini, K):
                    vector.tensor_scalar(
                        mini[tile][:, 4 * s : 4 * s + 4],
                        s4f[:],
                        mdf[:, s : s + 1],
                        None,
                        op0=A.is_equal,
                    )
                vector.drain().then_inc(off_sem, 1)

            def dense(tile):
                for j in range(JD[tile], K):
                    vector.tensor_scalar(
                        chunk_col(tile, j),
                        iotaf[:],
                        sf2[tile][:, K - 1 - j : K - j],
                        None,
                        op0=A.is_equal,
                    ).then_inc(pl_sem, 1)

            chain(0, gt0, in0_sem, 64, JD[0])
            chain(1, gt1, in1_sem, 32, JD[1])
            dense(0)
            vector.drain().then_inc(pl_sem, 1)
            dense(1)
            vector.drain().then_inc(pl_sem, 1)

    es.close()
    return nc


def _get_nc():
    if "nc" not in _CACHE:
        _CACHE["nc"] = _build_nc()
    return _CACHE["nc"]


def kernel(logits: np.ndarray, gn: np.ndarray) -> np.ndarray:
    from concourse.bass_utils import run_bass_kernel_spmd

    logits = np.ascontiguousarray(np.asarray(logits, dtype=np.float32))
    gn = np.asarray(gn, dtype=np.float32)
    assert logits.shape == (N, D) and gn.shape == (BS, N, D)

    nc = _get_nc()
    in_maps = []
    for c in range(NCORES):
        shard = np.ascontiguousarray(
            gn[c * BS_PER_CORE : (c + 1) * BS_PER_CORE].reshape(ROWS, D)
        )
        in_maps.append({"logits": logits, "gn": shard})

    res = run_bass_kernel_spmd(nc, in_maps, list(range(NCORES))).results
    out = np.concatenate(
        [r["out"].reshape(BS_PER_CORE, N, K, D) for r in res], axis=0
    )
    return out.astype(np.float32, copy=False)


# revision 3
# speedup vs baseline: 1.0823x; 1.0823x over previous
"""Trainium2 Bass kernel for nn_DPS_topk (topk_masking) — v8.

Forward output is exactly `hard`: the one-hot expansion of the top-16
indices of (logits + gn) along D, k-axis ordered by ascending index
(see v1 docstring for the stop_gradient cancellation proof).

One-hotness means 1023/1024 of output bytes are zeros, so HBM write
bandwidth binds.  v7 measured: the two HWDGE queues sustain ~440 GB/s
combined at plateau, but lost ~5us stalled on the tile-1 DVE chain
(DVE order was chain0/dense0/chain1/dense1) and ~1us on a late first
write (1 MiB DVE memset on the critical path).

v8 structure per core (256 rows = 2 tiles of 128, out = [256,16K] f32):

  - DVE: small memset (zeros[128,1024], ~0.9us) -> chain0 -> chain1 ->
    dense planes t0 j6..15 -> t1 j6..15.  Chains run back-to-back so
    every dense plane exists long before a queue reaches it.
  - per-plane zero DMAs (one plane = [128 rows, 1024 cols] = 512 KiB):
    tile0 j0..5 + tile1 j0..5 zero-filled; 5 planes on each HWDGE
    queue (sync/scalar), 2 on the gpsimd software queue.
  - ones via gpsimd indirect scatter (4-float granule, one offset per
    partition, ~1.25us desc-gen per plane-call), 12 calls, each gated
    on its plane's zero receipt + the tile's chain (off_sem).
  - dense planes j6..15 per tile (20 total) on DVE via one is_equal
    per plane, DMA'd from the two HWDGE queues gated at production
    index +1 (posted-write slack).
  - inputs: lg+gt0 halves lead each HWDGE queue (chain0 ready ~10us);
    gt1 loads ride the gpsimd software queue.

Raw Bass (no TileContext): one sync-wait condition per instruction;
explicit vector.drain() between dependent same-engine DVE ops; iota on
gpsimd (pattern steps, channel multiplier, base all HW-verified in v7).
"""

import numpy as np

K = 16
D = 1024
N = 64
BS = 32
NCORES = 8
BS_PER_CORE = BS // NCORES   # 4
ROWS = BS_PER_CORE * N       # 256 rows per core
P = 128                      # SBUF partitions
NTILES = ROWS // P           # 2
NELEM = ROWS * K * D
JD = [8, 8]                  # first dense plane per tile (zeros+scatter cover 0..JD-1)
NDENSE = (K - JD[0]) + (K - JD[1])   # 20 dense planes total

_CACHE = {}

# dense planes in DVE production order: t0 j6..15 (incs 1..10), drain (11),
# t1 j6..15 (incs 12..21), final drain (22).
DENSE_ORDER = [(0, j) for j in range(JD[0], K)] + [(1, j) for j in range(JD[1], K)]
ND0 = K - JD[0]
# DMA for a plane waits for the NEXT increment (one-op posted-write slack);
# the last plane of each tile is gated on that tile's drain.
PL_WAIT = {(0, j): (j - JD[0]) + 2 for j in range(JD[0], K - 1)}
PL_WAIT[(0, K - 1)] = ND0 + 1
PL_WAIT.update({(1, j): ND0 + 1 + (j - JD[1]) + 2 for j in range(JD[1], K - 1)})
PL_WAIT[(1, K - 1)] = ND0 + 2 + (K - JD[1])

# zero-plane queue assignment: 8 on sync (A), 8 on scalar (B)
ZPLANES_A = [(0, 0), (0, 2), (0, 4), (0, 6), (1, 1), (1, 3), (1, 5), (1, 7)]
ZPLANES_B = [(0, 1), (0, 3), (0, 5), (0, 7), (1, 0), (1, 2), (1, 4), (1, 6)]
ZPLANES_G = []
# dense-plane queue assignment (8 each, ascending pl-gate order)
DPLANES_A = [(0, 8), (0, 10), (0, 12), (0, 14),
             (1, 9), (1, 11), (1, 13), (1, 15)]
DPLANES_B = [(0, 9), (0, 11), (0, 13), (0, 15),
             (1, 8), (1, 10), (1, 12), (1, 14)]

# scatter order on gpsimd: tile0 j0..5 then tile1 j0..5.  Gate for (t,j):
# the receipt count on the zero-queue's sem at the position of that plane.
def _zgate(tile, j):
    for sem_name, lst in (("za", ZPLANES_A), ("zb", ZPLANES_B), ("zg", ZPLANES_G)):
        if (tile, j) in lst:
            return sem_name, 16 * (lst.index((tile, j)) + 1)
    raise KeyError((tile, j))


def _build_nc():
    from contextlib import ExitStack

    import concourse.bass as bass
    from concourse import mybir

    f32 = mybir.dt.float32
    i32 = mybir.dt.int32
    u32 = mybir.dt.uint32
    A = mybir.AluOpType

    nc = bass.Bass()
    lg_d = nc.declare_dram_parameter("logits", [N, D], f32, isOutput=False)
    gn_d = nc.declare_dram_parameter("gn", [ROWS, D], f32, isOutput=False)
    out_d = nc.declare_dram_parameter("out", [ROWS, K * D], f32, isOutput=True)

    es = ExitStack()

    def sb(name, shape, dt):
        return es.enter_context(nc.sbuf_tensor(name, shape, dt))

    def sem(name):
        return es.enter_context(nc.semaphore(name))

    zeros = sb("zeros", [P, D], f32)
    gt0 = sb("gt0", [P, D], f32)
    gt1 = sb("gt1", [P, D], f32)
    lg = sb("lg", [P, D], f32)
    x = sb("x", [P, D], f32)
    x2 = sb("x2", [P, D], f32)
    iotaf = sb("iotaf", [P, D], f32)
    chunk = sb("chunk", [P, NDENSE * D], f32)
    v8 = sb("v8", [P, 8], f32)
    v16 = sb("v16", [P, 8], f32)
    i8 = sb("i8", [P, 8], u32)
    i16 = sb("i16", [P, 8], u32)
    sf = sb("sf", [P, K], f32)
    sfr = sb("sfr", [P, K], f32)
    sf2 = [sb("sf2_0", [P, K], f32), sb("sf2_1", [P, K], f32)]
    idx32 = sb("idx32", [P, K], i32)
    al = sb("al", [P, K], i32)
    md = sb("md", [P, K], i32)
    mdf = sb("mdf", [P, K], f32)
    s4f = sb("s4f", [P, 4], f32)
    cm4 = sb("cm4", [P, 1], i32)
    c3 = sb("c3", [P, 1], i32)
    sb_base = [sb("sbase0", [P, K], i32), sb("sbase1", [P, K], i32)]
    offs = [sb("offs0", [P, K], i32), sb("offs1", [P, K], i32)]
    mini = [sb("mini0", [P, 4 * K], f32), sb("mini1", [P, 4 * K], f32)]

    zs_sem = sem("zs_sem")
    in0_sem = sem("in0_sem")
    in1_sem = sem("in1_sem")
    za_sem = sem("za_sem")
    zb_sem = sem("zb_sem")
    zg_sem = sem("zg_sem")
    pda_sem = sem("pda_sem")
    pdb_sem = sem("pdb_sem")
    gp_sem = sem("gp_sem")
    off_sem = sem("off_sem")
    pl_sem = sem("pl_sem")
    sc_sem = sem("sc_sem")
    SEMS = {"za": za_sem, "zb": zb_sem, "zg": zg_sem}

    with nc.Block(no_gpsimd_drain=True) as block:

        def plane_ap(tile, j):
            return bass.AP(out_d, tile * P * K * D + j * D, [[K * D, P], [1, D]])

        def chunk_col(tile, j):
            i = DENSE_ORDER.index((tile, j))
            return chunk[:, i * D : (i + 1) * D]

        @block.sync
        def _(sync: "bass.BassEngine"):
            sync.dma_start(out=lg[0:N, :], in_=lg_d[:, :]).then_inc(in0_sem, 16)
            sync.dma_start(out=gt0[0:N, :], in_=gn_d[0:N, :]).then_inc(in0_sem, 16)
            sync.dma_start(out=gt1[0:N, :], in_=gn_d[P : P + N, :]).then_inc(
                in1_sem, 16
            )
            sync.wait_ge(zs_sem, 1)
            for tile, j in ZPLANES_A:
                sync.dma_start(out=plane_ap(tile, j), in_=zeros[:, :]).then_inc(
                    za_sem, 16
                )
            for tile, j in DPLANES_A:
                sync.wait_ge(pl_sem, PL_WAIT[(tile, j)])
                sync.dma_start(
                    out=plane_ap(tile, j), in_=chunk_col(tile, j)
                ).then_inc(pda_sem, 16)
            sync.wait_ge(in0_sem, 64)
            sync.wait_ge(in1_sem, 32)
            sync.wait_ge(za_sem, 16 * len(ZPLANES_A))
            sync.wait_ge(zb_sem, 16 * len(ZPLANES_B))
            sync.wait_ge(pda_sem, 16 * len(DPLANES_A))
            sync.wait_ge(pdb_sem, 16 * len(DPLANES_B))

        @block.scalar
        def _(scalar: "bass.BassEngine"):
            scalar.dma_start(out=lg[N:P, :], in_=lg_d[:, :]).then_inc(in0_sem, 16)
            scalar.dma_start(out=gt0[N:P, :], in_=gn_d[N:P, :]).then_inc(
                in0_sem, 16
            )
            scalar.dma_start(out=gt1[N:P, :], in_=gn_d[P + N : 2 * P, :]).then_inc(
                in1_sem, 16
            )
            scalar.wait_ge(zs_sem, 1)
            for tile, j in ZPLANES_B:
                scalar.dma_start(out=plane_ap(tile, j), in_=zeros[:, :]).then_inc(
                    zb_sem, 16
                )
            for tile, j in DPLANES_B:
                scalar.wait_ge(pl_sem, PL_WAIT[(tile, j)])
                scalar.dma_start(
                    out=plane_ap(tile, j), in_=chunk_col(tile, j)
                ).then_inc(pdb_sem, 16)

        @block.gpsimd
        def _(gpsimd: "bass.BassEngine"):
            # scatter element offsets: elem(p, slot) =
            #   (tile*128+p)*16384 + (15-slot)*1024 + idx
            for i in range(NTILES):
                gpsimd.iota(
                    sb_base[i][:, :],
                    pattern=[[-D, K]],
                    base=i * P * K * D + (K - 1) * D,
                    channel_multiplier=K * D,
                )
            gpsimd.iota(s4f[:, :], pattern=[[1, 4]], base=0, channel_multiplier=0,
                        allow_small_or_imprecise_dtypes=True)
            gpsimd.iota(iotaf[:, :], pattern=[[1, D]], base=0, channel_multiplier=0,
                        allow_small_or_imprecise_dtypes=True)
            gpsimd.iota(cm4[:, :], pattern=[[1, 1]], base=-4, channel_multiplier=0)
            gpsimd.iota(c3[:, :], pattern=[[1, 1]], base=3, channel_multiplier=0)
            gpsimd.drain().then_inc(gp_sem, 1)

            def scall(tile, s):
                gpsimd.indirect_dma_start(
                    out=bass.AP(out_d, 0, [[1, NELEM], [1, 1]]),
                    out_offset=bass.IndirectOffsetOnAxis(
                        ap=offs[tile][:, s : s + 1], axis=0
                    ),
                    in_=mini[tile][:, 4 * s : 4 * s + 4],
                    in_offset=None,
                ).then_inc(sc_sem, 16)

            # tile-0 planes 0..JD-1 (slots 15..), then tile-1
            gpsimd.wait_ge(off_sem, 1)
            for j in range(0, JD[0]):
                gsem, gval = _zgate(0, j)
                gpsimd.wait_ge(SEMS[gsem], gval)
                scall(0, K - 1 - j)
            gpsimd.wait_ge(off_sem, 2)
            for j in range(0, JD[1]):
                gsem, gval = _zgate(1, j)
                gpsimd.wait_ge(SEMS[gsem], gval)
                scall(1, K - 1 - j)
            gpsimd.wait_ge(sc_sem, 16 * (JD[0] + JD[1]))

        @block.vector
        def _(vector: "bass.BassEngine"):
            def dr():
                vector.drain()

            vector.memset(zeros[:], 0.0)
            vector.drain().then_inc(zs_sem, 1)
            vector.wait_ge(gp_sem, 1)

            def chain(tile, gt, in_sem, in_tgt, nmini):
                vector.wait_ge(in_sem, in_tgt)
                vector.tensor_tensor(x[:], gt[:], lg[:], op=A.add)
                dr()
                vector.max(v8[:], x[:])
                dr()
                vector.max_index(i8[:], v8[:], x[:])
                vector.match_replace(x2[:], v8[:], x[:], -1e30)
                dr()
                vector.tensor_copy(sf[:, 0:8], i8[:])
                vector.max(v16[:], x2[:])
                dr()
                vector.max_index(i16[:], v16[:], x2[:])
                dr()
                vector.tensor_copy(sf[:, 8:16], i16[:])
                dr()
                vector.max(sf2[tile][:, 0:8], sf[:])
                dr()
                vector.match_replace(sfr[:], sf2[tile][:, 0:8], sf[:], -1.0)
                dr()
                vector.max(sf2[tile][:, 8:16], sfr[:])
                dr()
                vector.tensor_copy(idx32[:], sf2[tile][:])
                dr()
                vector.tensor_tensor(
                    al[:], idx32[:], cm4[:].to_broadcast([P, K]), op=A.bitwise_and
                )
                vector.tensor_tensor(
                    md[:], idx32[:], c3[:].to_broadcast([P, K]), op=A.bitwise_and
                )
                dr()
                vector.tensor_tensor(
                    offs[tile][:], al[:], sb_base[tile][:], op=A.add
                )
                vector.tensor_copy(mdf[:], md[:])
                dr()
                for s in range(K - nmini, K):
                    vector.tensor_scalar(
                        mini[tile][:, 4 * s : 4 * s + 4],
                        s4f[:],
                        mdf[:, s : s + 1],
                        None,
                        op0=A.is_equal,
                    )
                vector.drain().then_inc(off_sem, 1)

            def dense(tile):
                for j in range(JD[tile], K):
                    vector.tensor_scalar(
                        chunk_col(tile, j),
                        iotaf[:],
                        sf2[tile][:, K - 1 - j : K - j],
                        None,
                        op0=A.is_equal,
                    ).then_inc(pl_sem, 1)

            chain(0, gt0, in0_sem, 64, JD[0])
            chain(1, gt1, in1_sem, 32, JD[1])
            dense(0)
            vector.drain().then_inc(pl_sem, 1)
            dense(1)
            vector.drain().then_inc(pl_sem, 1)

    es.close()
    return nc


def _get_nc():
    if "nc" not in _CACHE:
        _CACHE["nc"] = _build_nc()
    return _CACHE["nc"]


def kernel(logits: np.ndarray, gn: np.ndarray) -> np.ndarray:
    from concourse.bass_utils import run_bass_kernel_spmd

    logits = np.ascontiguousarray(np.asarray(logits, dtype=np.float32))
    gn = np.asarray(gn, dtype=np.float32)
    assert logits.shape == (N, D) and gn.shape == (BS, N, D)

    nc = _get_nc()
    in_maps = []
    for c in range(NCORES):
        shard = np.ascontiguousarray(
            gn[c * BS_PER_CORE : (c + 1) * BS_PER_CORE].reshape(ROWS, D)
        )
        in_maps.append({"logits": logits, "gn": shard})

    res = run_bass_kernel_spmd(nc, in_maps, list(range(NCORES))).results
    out = np.concatenate(
        [r["out"].reshape(BS_PER_CORE, N, K, D) for r in res], axis=0
    )
    return out.astype(np.float32, copy=False)
